# revision 14
# baseline (speedup 1.0000x reference)
"""Multi-head attention (B=4, N=2048, C=1024, H=16, D=64) on 8 trn2 cores.

Sharding: core c handles batch b = c//2 and head-group g = c%2 (8 heads,
512 channels). No collectives: the row-parallel output projection partials
are summed on host (2 cores per batch), with bp + bv@Wp folded in on host
(softmax rows sum to 1). The k-bias is dropped entirely: it shifts every
key's score for a given query equally, which softmax normalizes away.

Device layout is transposed end-to-end (tokens on the free axis):
  xT [C, N] -> Q^T/K^T pair tiles [128, N] -> S^T tiles [keys, queries]
  (two K=64 matmuls row-packed on disjoint PE row groups, concurrent) ->
  exp (fp8e4 out) -> PV as fp8 DoubleRow over KEY-TILE PAIRS (256-key
  contraction per pass, 2x PE throughput) with V_aug [vA | ones | vB] ->
  DVE reciprocal * mul -> O^T -> Y^T = Wp_g^T O^T.

vs the 384us baseline:
  - QKV projections: each K=128 contraction tile runs as two concurrent
    K=64 row-tiled halves (PE row groups 0-1/2-3, separate PSUM banks);
    LDWEIGHTS of one half hides under the other's stream. Merge costs one
    ACT op (with Q-bias / dtype cast) + one DVE tensor_add.
  - PV: e and V_aug are fp8e4; DoubleRow contracts two key tiles per
    matmul. Accuracy (sim): ~1.6e-2 fro vs the 2e-2 gate, dominated by
    fp8 quantization of e and v; softmax normalization absorbs most of it.
  - exp split across engines: ACT native Exp -> fp8, a tunable subset on
    DVE as a Schraudolph bit-trick (i8 = sa*A + B through an int8 view).
  - Output projection: unpaired (serial-k) matmuls, single DVE evac; the
    final query-chunk's groups run on the then-idle S psum pool.

Matmul operands bf16 except the fp8 PV; accumulation fp32 in PSUM.
"""

import math
import os
import sys

sys.path.insert(0, "/opt/trn_rl_repo")

import numpy as np

B, N, C, H = 4, 2048, 1024, 16
D = C // H
SCALE = D**-0.5
NCORES = 8
FC = 512  # channels per core
NP = 4  # head pairs per core
KT8 = C // 128  # contraction tiles
NCQ = N // 512  # n-chunks of 512
NMT = N // 128  # key tiles
NDS = NMT // 2  # key-tile double-steps per strip

MM_DT = os.environ.get("KERNEL_MM_DT", "bf16")
PV_MODE = os.environ.get("KERNEL_PV", "dr")  # dr (fp8 DoubleRow) | bf16
# hybrid PV: double-steps with ds < DR_LIMIT use fp8 DoubleRow (2 key tiles
# per pass); the rest use plain fp8 matmuls (1 key tile each, bf16-rate)
DR_LIMIT = int(os.environ.get("KERNEL_DR_LIMIT", "8"))
# key-tiles of each strip whose exp runs on DVE (Schraudolph) vs ACT
DVE_MTS = frozenset(
    int(x) for x in os.environ.get("KERNEL_DVE_MTS", "2,5,8,11,14").split(",") if x != ""
)
# Schraudolph: i = round(sa * SCH_A + SCH_B); bits are the fp8e4 (DR mode)
# or bf16 (bf16 mode) of ~exp(sa*SCALE). HW DVE f32->int rounds to nearest.
if PV_MODE == "dr":
    SCH_A = (8.0 / math.log(2.0)) * SCALE
    SCH_B = 56.0 - 0.375 + float(os.environ.get("KERNEL_SCH_B_ADJ", "0"))
else:
    SCH_A = (128.0 / math.log(2.0)) * SCALE
    SCH_B = 127.0 * 128.0 - 5.5 + float(os.environ.get("KERNEL_SCH_B_ADJ", "0"))

_nc = None


def _ap3(ap_slice, block_step, nblocks, width, src_off=0):
    """3-dim AP: nblocks blocks of `width` cols, stride block_step."""
    import concourse.bass as bass

    lst = [list(p) for p in ap_slice.ap]
    assert len(lst) == 2 and lst[1][0] == 1, lst
    return bass.AP(
        ap_slice.tensor,
        ap_slice.offset + src_off,
        [lst[0], [block_step, nblocks], [1, width]],
    )


def _build():
    import concourse.bacc as bacc
    import concourse.mybir as mybir
    import concourse.tile as tile

    F32 = mybir.dt.float32
    MDT = mybir.dt.bfloat16 if MM_DT == "bf16" else mybir.dt.float32r
    EDT = mybir.dt.float8e4 if PV_MODE == "dr" else MDT
    SCH_I = mybir.dt.int8 if PV_MODE == "dr" else mybir.dt.int16
    AF = mybir.ActivationFunctionType
    ALU = mybir.AluOpType
    DR = mybir.MatmulPerfMode.DoubleRow

    nc = bacc.Bacc("TRN2", target_bir_lowering=False, debug=False, num_devices=NCORES)

    xT_d = nc.dram_tensor("xT", (NCQ, 128, KT8 * 512), MDT, kind="ExternalInput").ap()
    wq_d = nc.dram_tensor("wq", (128, KT8 * FC), MDT, kind="ExternalInput").ap()
    wk_d = nc.dram_tensor("wk", (128, KT8 * FC), MDT, kind="ExternalInput").ap()
    wv_d = nc.dram_tensor("wv", (128, KT8 * FC), MDT, kind="ExternalInput").ap()
    wp_d = nc.dram_tensor("wp", (128, NP * C), MDT, kind="ExternalInput").ap()
    bq_d = nc.dram_tensor("bq", (128, NP), F32, kind="ExternalInput").ap()
    yT_d = nc.dram_tensor("yT", (C // 128, NCQ, 128, 512), F32, kind="ExternalOutput").ap()

    with tile.TileContext(nc) as tc:
        with (
            tc.tile_pool(name="sb", bufs=1) as sb,
            tc.tile_pool(name="pe_", bufs=4) as pbe,
            tc.tile_pool(name="ptm", bufs=2) as ptm,
            tc.tile_pool(name="prc", bufs=4) as prc,
            tc.tile_pool(name="pyb", bufs=4) as pyb,
            tc.tile_pool(name="psa", bufs=2, space="PSUM") as psa,
            tc.tile_pool(name="psqk", bufs=1, space="PSUM") as psqk,
            tc.tile_pool(name="pso", bufs=2, space="PSUM") as pso,
        ):
            # ---- resident tiles + DMAs (order = priority) ----
            QT = [sb.tile([128, N], MDT, name=f"qt{p}") for p in range(NP)]
            KT = [sb.tile([128, N], MDT, name=f"kt{p}") for p in range(NP)]
            # V_aug for a key-tile PAIR: [tile-even 192*NP | tile-odd 192*NP] fp8
            VA = [sb.tile([128, 2 * 192 * NP], EDT, name=f"va{t2}") for t2 in range(NDS)]
            OT = [sb.tile([128, N], MDT, name=f"ot{p}") for p in range(NP)]
            bq_t = sb.tile([128, NP], F32, name="bq_t")
            nc.sync.dma_start(out=bq_t[:], in_=bq_d)
            for t2 in range(NDS):
                for par in range(2):
                    nc.vector.memset(
                        _ap3(VA[t2][:, 64:128], 192, NP, 64, src_off=768 * par), 1.0
                    )

            def _wall(nm, src_ap):
                t = sb.tile([128, KT8 * FC], MDT, name=nm)
                nc.sync.dma_start(out=t[:], in_=src_ap)
                return t

            def _xall(ncq):
                t = sb.tile([128, KT8 * 512], MDT, name=f"xt_{ncq}")
                nc.sync.dma_start(out=t[:], in_=xT_d[ncq, :, :])
                return t

            wq_all = _wall("wq_all", wq_d)
            xt_all = [_xall(0)]
            wk_all = _wall("wk_all", wk_d)
            wv_all = _wall("wv_all", wv_d)
            for ncq in range(1, NCQ):
                xt_all.append(_xall(ncq))
            wp_all = sb.tile([128, NP * C], MDT, name="wp_all")
            nc.sync.dma_start(out=wp_all[:], in_=wp_d)

            def mm_pair(pg, w, x, k, klast):
                """K=128 contraction tile as two concurrent K=64 row-tiled
                halves into separate PSUM banks (alpha cols 0:512, beta
                512:1024)."""
                nc.tensor.matmul(
                    pg[:, 0:512], w[0:64, :], x[0:64, :],
                    start=(k == 0), stop=klast, skip_group_check=True,
                )
                nc.tensor.matmul(
                    pg[:, 512:1024], w[64:128, :], x[64:128, :],
                    start=(k == 0), stop=klast, skip_group_check=True,
                )

            def emit_qk_group(p, proj, ncq, pool):
                w_all, dst = (wq_all, QT) if proj == 0 else (wk_all, KT)
                cs = slice(512 * ncq, 512 * (ncq + 1))
                state = {}

                def mk_slot(k):
                    def go():
                        if "pg" not in state:
                            state["pg"] = pool.tile(
                                [128, 1024], F32, tag="g", name=f"pg_{p}_{proj}_{ncq}"
                            )
                        mm_pair(
                            state["pg"],
                            w_all[:, FC * k + 128 * p : FC * k + 128 * (p + 1)],
                            xt_all[ncq][:, 512 * k : 512 * (k + 1)],
                            k, k == KT8 - 1,
                        )

                    return go

                def ev1():
                    if proj == 0:
                        nc.scalar.add(dst[p][:, cs], state["pg"][:, 0:512], bq_t[:, p : p + 1])
                    else:
                        nc.vector.tensor_copy(dst[p][:, cs], state["pg"][:, 0:512])

                def ev2():
                    nc.vector.tensor_add(
                        dst[p][:, cs], dst[p][:, cs], state["pg"][:, 512:1024]
                    )

                return [mk_slot(k) for k in range(KT8)] + [ev1, ev2]

            def emit_v_group(nt, pool):
                """V projection for key tile nt -> fp8 V_aug blocks. alpha is
                staged through a bf16 tmp so v is fp8-rounded only once."""
                ncq, t = divmod(nt, 4)
                t2, par = divmod(nt, 2)
                state = {}

                def mk_slot(k):
                    def go():
                        if "pg" not in state:
                            state["pg"] = pool.tile(
                                [128, 1024], F32, tag="g", name=f"pv_{nt}"
                            )
                        mm_pair(
                            state["pg"],
                            xt_all[ncq][:, 512 * k + 128 * t : 512 * k + 128 * (t + 1)],
                            wv_all[:, FC * k : FC * (k + 1)],
                            k, k == KT8 - 1,
                        )

                    return go

                def ev1():
                    tm = ptm.tile([128, 512], F32, tag="tm", name=f"vtm_{nt}")
                    state["tm"] = tm
                    nc.scalar.copy(tm[:], state["pg"][:, 0:512])

                def ev2(half):
                    # half 0: vA blocks -> va cols 0:64 (+192k); half 1: vB ->
                    # cols 128:192. out fp8 = tmp(alpha) + beta.
                    dst = _ap3(
                        VA[t2][:, 0:64] if half == 0 else VA[t2][:, 128:192],
                        192, NP, 64, src_off=768 * par,
                    )
                    a = _ap3(state["tm"][:, 0:64], 128, NP, 64, src_off=64 * half)
                    b = _ap3(state["pg"][:, 512:576], 128, NP, 64, src_off=64 * half)
                    nc.vector.tensor_add(dst, a, b)

                return [mk_slot(k) for k in range(KT8)] + [
                    ev1, lambda: ev2(0), lambda: ev2(1),
                ]

            def emit_proj_group(c, ncq, pool):
                """Y^T chunk: 4 serial accumulating matmuls + DVE evac + DMA."""
                cs = slice(512 * ncq, 512 * (ncq + 1))
                state = {}

                def mk_slot(f):
                    def go():
                        if "pg" not in state:
                            state["pg"] = pool.tile(
                                [128, 1024], F32, tag="g", name=f"py_{c}_{ncq}"
                            )
                        nc.tensor.matmul(
                            state["pg"][:, 0:512],
                            wp_all[:, C * f + 128 * c : C * f + 128 * (c + 1)],
                            OT[f][:, cs],
                            start=(f == 0), stop=(f == NP - 1), skip_group_check=True,
                        )

                    return go

                def ev():
                    yb = pyb.tile([128, 512], F32, tag="yb", name=f"yb_{c}_{ncq}")
                    nc.vector.tensor_copy(yb[:], state["pg"][:, 0:512])
                    nc.sync.dma_start(out=yT_d[c, ncq, :, :], in_=yb[:])

                return [mk_slot(f) for f in range(NP)] + [ev]

            # ---- prologue: K all chunks + Q chunk 0 + all V, in DMA order;
            # groups rotate over 3 psum slots (psa x2 + psqk) so evac chains
            # hide behind the next group's compute ----
            pro = []
            pro.append(("qk", 0, 0, 0))
            pro.append(("qk", 0, 1, 0))
            for t in range(4):
                pro.append(("v", 4 * 0 + t,))
            for ncq in range(1, NCQ):
                if ncq == 1:
                    pro.append(("qk", 0, 1, ncq))  # K chunks 2-3 go to filler
                for t in range(4):
                    pro.append(("v", 4 * ncq + t,))
            for gi, g in enumerate(pro):
                pool = psqk if gi % 3 == 2 else psa
                if g[0] == "qk":
                    for go in emit_qk_group(g[1], g[2], g[3], pool):
                        go()
                else:
                    for go in emit_v_group(g[1], pool):
                        go()

            # ---- attention strips ----
            filler = []
            steps = [(p, qc, mt) for p in range(NP) for qc in range(NCQ) for mt in range(NMT)]

            def emit_S(p, qc, mt):
                qs = slice(512 * qc, 512 * (qc + 1))
                ms = slice(128 * mt, 128 * (mt + 1))
                sa = psa.tile([128, 1024], F32, tag="g", name=f"sa_{p}_{qc}_{mt}")
                nc.tensor.matmul(
                    sa[:, 0:512], KT[p][0:64, ms], QT[p][0:64, qs], start=True, stop=True
                )
                nc.tensor.matmul(
                    sa[:, 512:1024], KT[p][64:128, ms], QT[p][64:128, qs],
                    start=True, stop=True,
                )
                return sa

            ots, ea2 = None, None
            sa_next = emit_S(*steps[0])
            for i, (p, qc, mt) in enumerate(steps):
                if mt == 0:
                    if qc == 0:
                        if p == 0:
                            for ncq in range(2, NCQ):
                                filler.extend(emit_qk_group(0, 1, ncq, psqk))
                            for ncq in range(1, NCQ):
                                filler.extend(emit_qk_group(0, 0, ncq, psqk))
                        if p + 1 < NP:
                            for proj in range(2):
                                for ncq in range(NCQ):
                                    filler.extend(emit_qk_group(p + 1, proj, ncq, psqk))
                    ots = [
                        pso.tile([128, 512], F32, tag="o", name=f"o_{p}_{qc}_{j}")
                        for j in range(2)
                    ]
                ds, par = divmod(mt, 2)
                if par == 0:
                    ea2 = pbe.tile([128, 2048], EDT, tag="e", name=f"ea_{p}_{qc}_{ds}")
                sa_cur = sa_next
                dst = ea2[:, 1024 * par : 1024 * (par + 1)]
                if mt in DVE_MTS:
                    nc.vector.tensor_scalar(
                        dst.bitcast(SCH_I), sa_cur[:], SCH_A, SCH_B, ALU.mult, ALU.add
                    )
                else:
                    nc.scalar.activation(dst, sa_cur[:], AF.Exp, scale=SCALE)
                if i + 1 < len(steps):
                    sa_next = emit_S(*steps[i + 1])
                for _ in range(4 if p == NP - 1 else 2):
                    if filler:
                        filler.pop(0)()
                if PV_MODE == "dr":
                    if ds < DR_LIMIT:
                        if par == 1:
                            first = ds == 0
                            last = DR_LIMIT >= NDS and ds == NDS - 1
                            # fp8 DoubleRow: contract both key tiles of the pair
                            # in one pass; lhsT [128,(2),128] interleaves the two
                            # V_aug tiles, rhs [128,(2),512] the two e tiles.
                            for j in range(2):
                                woff = 192 * p + (0 if j == 0 else 64)
                                nc.tensor.matmul(
                                    ots[j],
                                    _ap3(VA[ds][:, woff : woff + 128], 768, 2, 128),
                                    _ap3(ea2[:, 512 * j : 512 * j + 512], 1024, 2, 512),
                                    start=first, stop=last, skip_group_check=True,
                                    perf_mode=DR,
                                )
                    else:
                        first = ds == 0 and par == 0 and DR_LIMIT == 0
                        last = mt == NMT - 1
                        for j in range(2):
                            woff = 768 * par + 192 * p + (0 if j == 0 else 64)
                            nc.tensor.matmul(
                                ots[j],
                                VA[ds][:, woff : woff + 128],
                                ea2[:, 1024 * par + 512 * j : 1024 * par + 512 * (j + 1)],
                                start=first, stop=last, skip_group_check=True,
                            )
                else:
                    first, last = mt == 0, mt == NMT - 1
                    for j in range(2):
                        woff = 768 * par + 192 * p + (0 if j == 0 else 64)
                        nc.tensor.matmul(
                            ots[j],
                            VA[ds][:, woff : woff + 128],
                            ea2[:, 1024 * par + 512 * j : 1024 * par + 512 * (j + 1)],
                            start=first, stop=last, skip_group_check=True,
                        )
                if mt == NMT - 1:
                    qs = slice(512 * qc, 512 * (qc + 1))
                    for j in range(2):
                        o = ots[j]
                        # reciprocal_approx_fast mis-executes at base partition
                        # != 0: run over the whole tile, slice after.
                        rc = prc.tile([128, 512], F32, tag="rc", name=f"rc_{p}_{qc}_{j}")
                        nc.vector.reciprocal_approx_fast(rc[:], o[:])
                        osl, rcl = (
                            (o[0:64, :], rc[64:128, :]) if j == 0 else (o[64:128, :], rc[0:64, :])
                        )
                        nc.vector.tensor_mul(OT[p][64 * j : 64 * j + 64, qs], osl, rcl)
                    if p == NP - 1:
                        # final query-chunk's projection runs on the S pool,
                        # idle once the last exp is done
                        pool = psa if qc == NCQ - 1 else psqk
                        for c in range(C // 128):
                            filler.extend(emit_proj_group(c, qc, pool))
            while filler:
                filler.pop(0)()

    nc.compile()
    return nc


def _get_nc():
    global _nc
    if _nc is None:
        try:
            import jax

            jax.config.update(
                "jax_compilation_cache_dir", os.path.expanduser("~/.cache/jax_bass")
            )
            jax.config.update("jax_persistent_cache_min_compile_time_secs", 0.0)
            jax.config.update("jax_persistent_cache_min_entry_size_bytes", 0)
        except Exception:
            pass
        _nc = _build()
    return _nc


def _wmerge(w, mdt):
    """(KT*128, F) -> [128, KT*F] partition-major merged layout."""
    kt = w.shape[0] // 128
    return np.ascontiguousarray(
        w.reshape(kt, 128, w.shape[1]).transpose(1, 0, 2).reshape(128, kt * w.shape[1]).astype(mdt)
    )


def make_in_maps(inputs):
    if MM_DT == "bf16":
        import ml_dtypes

        mdt = ml_dtypes.bfloat16
    else:
        mdt = np.float32
    x = np.asarray(inputs["x"], np.float32)
    Wq = np.asarray(inputs["Wq"], np.float32)
    Wk = np.asarray(inputs["Wk"], np.float32)
    Wv = np.asarray(inputs["Wv"], np.float32)
    Wp = np.asarray(inputs["Wp"], np.float32)
    bq = np.asarray(inputs["bq"], np.float32)
    in_maps = []
    for core in range(NCORES):
        b, g = core // 2, core % 2
        sl = slice(FC * g, FC * (g + 1))
        in_maps.append(
            {
                "xT": np.ascontiguousarray(
                    x[b].T.reshape(KT8, 128, NCQ, 512)
                    .transpose(2, 1, 0, 3)
                    .reshape(NCQ, 128, KT8 * 512)
                    .astype(mdt)
                ),
                "wq": _wmerge(Wq[:, sl], mdt),
                "wk": _wmerge(Wk[:, sl], mdt),
                "wv": _wmerge(Wv[:, sl], mdt),
                "wp": _wmerge(Wp[sl, :], mdt),
                "bq": np.ascontiguousarray(bq[sl].reshape(NP, 128).T),
            }
        )
    return in_maps


def assemble(results, inputs):
    Wp = np.asarray(inputs["Wp"], np.float32)
    bv = np.asarray(inputs["bv"], np.float32)
    bp = np.asarray(inputs["bp"], np.float32)
    fb = (bp.astype(np.float64) + bv.astype(np.float64) @ Wp.astype(np.float64)).astype(
        np.float32
    )
    out = np.empty((B, N, C), np.float32)
    for b in range(B):
        yt = (results[2 * b]["yT"] + results[2 * b + 1]["yT"]).transpose(0, 2, 1, 3)
        out[b] = yt.reshape(C, N).T + fb
    return out


def run_on_device(inputs, trace=False, tmpdir=None):
    from concourse.bass_utils import run_bass_kernel_spmd

    nc = _get_nc()
    res = run_bass_kernel_spmd(
        nc, make_in_maps(inputs), list(range(NCORES)), trace=trace, tmpdir=tmpdir
    )
    return assemble(res.results, inputs), res


def kernel(**inputs):
    out, _ = run_on_device(inputs)
    return out
